# revision 1
# baseline (speedup 1.0000x reference)
"""DGCNN forward on 8 Trainium2 NeuronCores, data-parallel over batch.

Per core (one point cloud, x [3, 2048]):
  4 edge-conv blocks, each:
    s[n,m] = 2*x_n.x_m - |x_m|^2            (augmented fp32 PE matmul; row-constant
                                             -|x_n|^2 dropped: rank-invariant per row)
    top-20 of each s row:  segment-max(8) -> top-20 segments (DVE max8/max_index/
    match_replace) -> candidate values bounced через DRAM (indirect DMA, per-
    partition offsets) -> exact top-20 + global ids
    x' = lrelu(max_k A[:, idx_k] + B)       (A = Wn@x, B = (Wc-Wn)@x; edge features
                                             never materialized: conv is linear and
                                             lrelu/max commute)
  then g = lrelu(Wg @ concat(x1..x4)), out = max_n g.

All matmuls exact fp32 (fp32r's ~1e-3 noise provably corrupts the knn sets).
"""

import numpy as np
from contextlib import ExitStack

import concourse.bass as bass
import concourse.bacc as bacc
import concourse.mybir as mybir
from concourse import library_config
from concourse.bass import IndirectOffsetOnAxis
from concourse.tile import TileContext

F32 = mybir.dt.float32
U16 = mybir.dt.uint16
I16 = mybir.dt.int16
U32 = mybir.dt.uint32

B, N, KNN, P = 8, 2048, 20, 128
NCHUNK = N // P            # 16
SEG = 8                    # segment size for the topk pre-filter
NSEG = N // SEG            # 256 segments per row
NCAND = KNN * SEG          # 160 candidate values per row
NEG = -3.0e38
SLOPE = 0.2
BLOCKS = [(3, 64), (64, 64), (64, 128), (128, 256)]
ACT = mybir.ActivationFunctionType


import os
_DBG_STAGE = os.environ.get("DGCNN_DEBUG_STAGE", "")


def _ceil(a, b):
    return (a + b - 1) * b // b


def build_nc():
    nc = bacc.Bacc("TRN2", target_bir_lowering=False)

    x_in = nc.dram_tensor("x", [3, N], F32, kind="ExternalInput")
    w_in = {}
    for bi, (c, o) in enumerate(BLOCKS):
        w_in[f"wnt{bi}"] = nc.dram_tensor(f"wnt{bi}", [c, o], F32, kind="ExternalInput")
        w_in[f"wdt{bi}"] = nc.dram_tensor(f"wdt{bi}", [c, o], F32, kind="ExternalInput")
    w_in["wgt"] = nc.dram_tensor("wgt", [512, 1024], F32, kind="ExternalInput")
    id_in = nc.dram_tensor("ident", [P, P], F32, kind="ExternalInput")
    out_d = nc.dram_tensor("out", [1024, 1], F32, kind="ExternalOutput")
    if _DBG_STAGE == "dump":
        dbg_idx = nc.dram_tensor("dbg_idx", [P, 24], U16, kind="ExternalOutput")
        dbg_gath = nc.dram_tensor("dbg_gath", [P, KNN * 64], F32, kind="ExternalOutput")
        dbg_x1 = nc.dram_tensor("dbg_x1", [64, N], F32, kind="ExternalOutput")
        dbg_x2 = nc.dram_tensor("dbg_x2", [64, N], F32, kind="ExternalOutput")
        dbg_x3 = nc.dram_tensor("dbg_x3", [P, N], F32, kind="ExternalOutput")
        dbg_x4 = nc.dram_tensor("dbg_x4", [P, N], F32, kind="ExternalOutput")

    # DRAM scratch: per-block A^T feature tables
    at_dram = [
        nc.dram_tensor(f"at{bi}_scratch", [N, o], F32, kind="Internal")
        for bi, (_, o) in enumerate(BLOCKS)
    ]

    with TileContext(nc) as tc, ExitStack() as ctx:
        ep = ctx.enter_context            # shorthand
        const = ep(tc.tile_pool(name="const", bufs=1))
        wpool = ep(tc.tile_pool(name="weights", bufs=1))
        xpool = ep(tc.tile_pool(name="xtiles", bufs=1))
        bpool = ep(tc.tile_pool(name="blockp", bufs=1))
        spool = ep(tc.tile_pool(name="sbuf", bufs=2))
        tkpool = ep(tc.tile_pool(name="topk", bufs=2))
        gpool = ep(tc.tile_pool(name="gather", bufs=2))
        pp_s = ep(tc.tile_pool(name="ps_s", bufs=2, space="PSUM"))      # [128,1024] x2 -> 4 banks
        pp_m = ep(tc.tile_pool(name="ps_m", bufs=2, space="PSUM"))      # [128,<=512] -> 2 banks
        pp_t = ep(tc.tile_pool(name="ps_t", bufs=2, space="PSUM"))      # [128,128]   -> 2 banks

        # ---- constants ----
        ident = const.tile([P, P], F32)
        nc.sync.dma_start(out=ident[:], in_=id_in[:])
        ones_col = const.tile([P, 1], F32)          # lhsT for xx row-sum matmul
        nc.vector.memset(ones_col[:], 1.0)
        ones_row = const.tile([1, P], F32)          # lhsT for the -xx gram row (block4)
        nc.vector.memset(ones_row[:], 1.0)

        # ---- load pre-transposed weights (host supplies WnT/WdT/WgT) ----
        wnT, wdT = [], []
        for bi, (c, o) in enumerate(BLOCKS):
            wn = wpool.tile([c, o], F32, tag=f"wn{bi}")
            nc.sync.dma_start(out=wn[:], in_=w_in[f"wnt{bi}"][:])
            wd = wpool.tile([c, o], F32, tag=f"wd{bi}")
            nc.sync.dma_start(out=wd[:], in_=w_in[f"wdt{bi}"][:])
            wnT.append(wn)
            wdT.append(wd)

        # WgT: [512, 1024] as 4 chunk tiles [128, 1024]
        wgT = [wpool.tile([P, 1024], F32, tag=f"wg{k}", name=f"wgT{k}") for k in range(4)]
        for k in range(4):
            nc.sync.dma_start(out=wgT[k][:], in_=w_in["wgt"][k * P:(k + 1) * P, :])

        # ---- x_cat tiles (c12 assembled from x1t/x2t before the global conv) ----
        c12 = xpool.tile([P, N], F32)    # rows 0:64 = x1, 64:128 = x2
        c3 = xpool.tile([P, N], F32)
        c4a = xpool.tile([P, N], F32)
        c4b = xpool.tile([P, N], F32)
        x1t = xpool.tile([64, N], F32)
        x2t = xpool.tile([64, N], F32)

        # block input x0
        x0 = xpool.tile([3, N], F32)
        nc.sync.dma_start(out=x0[:], in_=x_in[:])

        def block_input(bi):
            return [x0[:], x1t[:], x2t[:], c3[:]][bi]

        def block_out_dst(bi):
            # list of dst tiles per 128-row output slab
            return [[x1t], [x2t], [c3], [c4a, c4b]][bi]

        # ================= edge blocks =================
        for bi, (C, O) in enumerate(BLOCKS):
            xb = block_input(bi)                       # [C, N] fp32 SBUF

            # --- per-block prep: x2 = 2x, negxx row, A^T table to DRAM ---
            x2 = bpool.tile([C, N], F32, tag="x2")
            nc.scalar.activation(out=x2[:], in_=xb, func=ACT.Copy, scale=2.0)
            xsq = bpool.tile([C, N], F32, tag="xsq")
            nc.scalar.activation(out=xsq[:], in_=xb, func=ACT.Square)
            negxx = bpool.tile([1, N], F32, tag="negxx")
            for q in range(4):
                mq = pp_m.tile([1, 512], F32, space="PSUM", tag="mm")
                nc.tensor.matmul(out=mq[:], lhsT=ones_col[:C, :], rhs=xsq[:, q * 512:(q + 1) * 512],
                                 start=True, stop=True)
                nc.scalar.activation(out=negxx[:, q * 512:(q + 1) * 512], in_=mq[:],
                                     func=ACT.Copy, scale=-1.0)

            # A^T rows to DRAM: at[n, :] = x_n . WnT  (chunk-wise)
            for i in range(NCHUNK):
                ap_ = pp_m.tile([P, O], F32, space="PSUM", tag="mm")
                nc.tensor.matmul(out=ap_[:], lhsT=xb[:, i * P:(i + 1) * P], rhs=wnT[bi][:],
                                 start=True, stop=True)
                at_sb = spool.tile([P, O], F32, tag="at_sb")
                nc.scalar.copy(out=at_sb[:], in_=ap_[:])
                nc.sync.dma_start(out=at_dram[bi][i * P:(i + 1) * P, :], in_=at_sb[:])

            # augmented gram lhs/rhs for C <= 64 (single fused matmul); block4 separate
            if C <= 64:
                # augmented row must start at a 32-aligned partition; pad with zeros
                cpad = ((C + 31) // 32) * 32
                gl = bpool.tile([cpad + 1, N], F32, tag="gramL")
                gr = bpool.tile([cpad + 1, N], F32, tag="gramR")
                if cpad != C:
                    nc.vector.memset(gl[:], 0.0)
                    nc.vector.memset(gr[:], 0.0)
                nc.scalar.copy(out=gl[:C, :], in_=xb)
                nc.vector.memset(gl[cpad:cpad + 1, :], 1.0)
                nc.vector.tensor_copy(out=gr[:C, :], in_=x2[:])
                # engines are lane-aligned: partition 0 -> cpad needs a DMA
                nc.sync.dma_start(out=gr[cpad:cpad + 1, :], in_=negxx[:])

            # --- main chunk loop ---
            for i in range(NCHUNK):
                # gram s[n, m] for n in chunk i : two psum halves [128, 1024]
                ps_h = []
                for h in range(2):
                    ph = pp_s.tile([P, 1024], F32, space="PSUM")
                    for q in range(2):
                        sl = slice((2 * h + q) * 512, (2 * h + q + 1) * 512)
                        if C <= 64:
                            nc.tensor.matmul(out=ph[:, q * 512:(q + 1) * 512],
                                             lhsT=gl[:, i * P:(i + 1) * P], rhs=gr[:, sl],
                                             start=True, stop=True)
                        else:
                            nc.tensor.matmul(out=ph[:, q * 512:(q + 1) * 512],
                                             lhsT=xb[:, i * P:(i + 1) * P], rhs=x2[:, sl],
                                             start=True, stop=False)
                            nc.tensor.matmul(out=ph[:, q * 512:(q + 1) * 512],
                                             lhsT=ones_row[:, :P],
                                             rhs=negxx[:, sl],
                                             start=False, stop=True)
                    ps_h.append(ph)

                # s -> SBUF (full row needed by the match-op scans)
                s_sb = tkpool.tile([P, N], F32, tag="s_sb")
                for h in range(2):
                    nc.scalar.copy(out=s_sb[:, h * 1024:(h + 1) * 1024], in_=ps_h[h][:])

                # exact top-20 of each row: 3 rounds of (max8, max_index,
                # in-place match_replace); s_sb is consumed
                v8 = tkpool.tile([P, 8], F32, tag="v8")
                i24 = tkpool.tile([P, 24], U16, tag="i24")
                nc.vector.max(out=v8[:], in_=s_sb[:])
                nc.vector.max_index(out=i24[:, 0:8], in_max=v8[:], in_values=s_sb[:])
                nc.vector.match_replace(out=s_sb[:], in_to_replace=v8[:], in_values=s_sb[:],
                                        imm_value=NEG)
                nc.vector.max(out=v8[:], in_=s_sb[:])
                nc.vector.max_index(out=i24[:, 8:16], in_max=v8[:], in_values=s_sb[:])
                nc.vector.match_replace(out=s_sb[:], in_to_replace=v8[:], in_values=s_sb[:],
                                        imm_value=NEG)
                nc.vector.max(out=v8[:], in_=s_sb[:])
                nc.vector.max_index(out=i24[:, 16:24], in_max=v8[:], in_values=s_sb[:])

                # gather neighbor A^T rows (one indirect DMA per k, per-partition
                # row offsets) and reduce max over k
                nbr32 = tkpool.tile([P, KNN], U32, tag="nbr32")
                nc.vector.tensor_copy(out=nbr32[:], in_=i24[:, :KNN])
                mx = gpool.tile([P, O], F32, tag="mx")
                if _DBG_STAGE == "nogather":
                    nc.vector.memset(mx[:], 0.0)
                else:
                    gath = gpool.tile([P, KNN, O], F32, tag="gath")
                    for j in range(KNN):
                        nc.gpsimd.indirect_dma_start(
                            out=gath[:, j, :], out_offset=None, in_=at_dram[bi][:],
                            in_offset=IndirectOffsetOnAxis(ap=nbr32[:, j:j + 1], axis=0))
                    nc.vector.tensor_reduce(
                        out=mx[:], in_=gath[:].rearrange("p k o -> p o k"),
                        axis=mybir.AxisListType.X, op=mybir.AluOpType.max)
                    if _DBG_STAGE == "dump" and bi == 0 and i == 0:
                        nc.sync.dma_start(out=dbg_idx[:], in_=i24[:])
                        nc.sync.dma_start(out=dbg_gath[:],
                                          in_=gath[:].rearrange("p k o -> p (k o)"))

                # B^T chunk, add, leaky relu
                bt = pp_m.tile([P, O], F32, space="PSUM", tag="mm")
                nc.tensor.matmul(out=bt[:], lhsT=xb[:, i * P:(i + 1) * P], rhs=wdT[bi][:],
                                 start=True, stop=True)
                xt = gpool.tile([P, O], F32, tag="xt")
                nc.vector.tensor_add(out=xt[:], in0=mx[:], in1=bt[:])
                # leaky relu = max(x, 0.2*x)  (Lrelu not modeled in CoreSim)
                xs = gpool.tile([P, O], F32, tag="xs")
                nc.vector.tensor_scalar_mul(xs[:], xt[:], SLOPE)
                nc.vector.tensor_tensor(out=xt[:], in0=xt[:], in1=xs[:],
                                        op=mybir.AluOpType.max)

                # transpose back to [O, chunk] into the x_cat tiles
                dsts = block_out_dst(bi)
                for q in range((O + P - 1) // P):
                    osz = min(P, O - q * P)
                    tp = pp_t.tile([P, P], F32, space="PSUM")
                    nc.tensor.transpose(out=tp[:osz, :], in_=xt[:, q * P:q * P + osz],
                                        identity=ident[:])
                    nc.scalar.copy(out=dsts[q][0:osz, i * P:(i + 1) * P], in_=tp[:osz, :])

        # ================= global conv + max =================
        if _DBG_STAGE == "dump":
            nc.sync.dma_start(out=dbg_x1[:], in_=x1t[:])
            nc.sync.dma_start(out=dbg_x2[:], in_=x2t[:])
            nc.sync.dma_start(out=dbg_x3[:], in_=c3[:])
            nc.sync.dma_start(out=dbg_x4[:], in_=c4a[:])
        nc.sync.dma_start(out=c12[0:64, :], in_=x1t[:])
        nc.sync.dma_start(out=c12[64:128, :], in_=x2t[:])
        xcat = [c12, c3, c4a, c4b]
        for oc in range(8):
            redv = spool.tile([P, 4], F32, tag="redv")
            for nq in range(4):
                pg = pp_s.tile([P, 512], F32, space="PSUM", tag="ph")
                for k in range(4):
                    nc.tensor.matmul(out=pg[:], lhsT=wgT[k][:, oc * P:(oc + 1) * P],
                                     rhs=xcat[k][:, nq * 512:(nq + 1) * 512],
                                     start=(k == 0), stop=(k == 3))
                nc.vector.tensor_reduce(out=redv[:, nq:nq + 1], in_=pg[:],
                                        axis=mybir.AxisListType.X, op=mybir.AluOpType.max)
            red1 = spool.tile([P, 1], F32, tag="red1")
            nc.vector.tensor_reduce(out=red1[:], in_=redv[:],
                                    axis=mybir.AxisListType.X, op=mybir.AluOpType.max)
            reds = spool.tile([P, 1], F32, tag="reds")
            nc.vector.tensor_scalar_mul(reds[:], red1[:], SLOPE)
            nc.vector.tensor_tensor(out=red1[:], in0=red1[:], in1=reds[:],
                                    op=mybir.AluOpType.max)
            nc.sync.dma_start(out=out_d[oc * P:(oc + 1) * P, :], in_=red1[:])

    nc.compile()
    return nc


_NC_CACHE = None


def _get_nc():
    global _NC_CACHE
    if _NC_CACHE is None:
        _NC_CACHE = build_nc()
    return _NC_CACHE


def host_weights(W1, W2, W3, W4, Wg):
    ws = {}
    for bi, (wm, (c, o)) in enumerate(zip([W1, W2, W3, W4], BLOCKS)):
        wm = np.asarray(wm, dtype=np.float32)
        wn = wm[:, :c]
        wd = wm[:, c:] - wn
        ws[f"wnt{bi}"] = np.ascontiguousarray(wn.T)
        ws[f"wdt{bi}"] = np.ascontiguousarray(wd.T)
    ws["wgt"] = np.ascontiguousarray(np.asarray(Wg, dtype=np.float32).T)
    ws["ident"] = np.eye(P, dtype=np.float32)
    return ws


def kernel(x, W1, W2, W3, W4, Wg):
    from concourse.bass_utils import run_bass_kernel_spmd

    nc = _get_nc()
    x = np.asarray(x, dtype=np.float32)
    ws = host_weights(W1, W2, W3, W4, Wg)
    in_maps = [{"x": np.ascontiguousarray(x[b]), **ws} for b in range(B)]
    res = run_bass_kernel_spmd(nc, in_maps, core_ids=list(range(B)))
    outs = res.results if hasattr(res, "results") else res
    return np.stack([outs[b]["out"].reshape(1024) for b in range(B)], axis=0)



# revision 22
# speedup vs baseline: 1.0651x; 1.0651x over previous
"""DGCNN forward on 8 Trainium2 NeuronCores, data-parallel over batch.

Per core (one point cloud, x [3, 2048]):
  4 edge-conv blocks, each:
    s[n,m] = 2*x_n.x_m - |x_m|^2            (fp32 PE matmul; row-constant
                                             -|x_n|^2 dropped: rank-invariant per row)
    exact top-20 of each s row: 3 rounds of (max8, max_index, match_replace) on DVE
    x' = lrelu(max_k A[:, idx_k] + B)       (A = Wn@x, B = (Wc-Wn)@x; edge features
                                             never materialized: conv is linear and
                                             lrelu/max commute)
  then g = lrelu(Wg @ concat(x1..x4)), out = max_n g.

Schedule: 3-stage software pipeline per block (gram+copies one chunk ahead,
topk+gather current, reduce/tail two chunks behind) so the in-order engine
sequencers never head-block on the 20 indirect gather DMAs (994 ns fixed
SWDGE overhead each, the Pool-engine pacer).  Each block's per-column prep
(2x, sum-of-squares row, next A^T table chunk) is fused into the previous
block's tail stage; the global conv runs interleaved with block 4.

All matmuls exact fp32 (fp32r's ~1e-3 noise corrupts the knn sets).
"""

import numpy as np
from contextlib import ExitStack

import concourse.bass as bass
import concourse.bacc as bacc
import concourse.mybir as mybir
from concourse.bass import IndirectOffsetOnAxis
from concourse.tile import TileContext

F32 = mybir.dt.float32
U16 = mybir.dt.uint16
U32 = mybir.dt.uint32

B, N, KNN, P = 8, 2048, 20, 128
NCHUNK = N // P            # 16
NEG = -3.0e38
SLOPE = 0.2
BLOCKS = [(3, 64), (64, 64), (64, 128), (128, 256)]
ACT = mybir.ActivationFunctionType


def build_nc():
    nc = bacc.Bacc("TRN2", target_bir_lowering=False)

    x_in = nc.dram_tensor("x", [3, N], F32, kind="ExternalInput")
    w_in = {}
    for bi, (c, o) in enumerate(BLOCKS):
        w_in[f"wnt{bi}"] = nc.dram_tensor(f"wnt{bi}", [c, o], F32, kind="ExternalInput")
        w_in[f"wdt{bi}"] = nc.dram_tensor(f"wdt{bi}", [c, o], F32, kind="ExternalInput")
    w_in["wgt"] = nc.dram_tensor("wgt", [512, 1024], F32, kind="ExternalInput")
    id_in = nc.dram_tensor("ident", [P, P], F32, kind="ExternalInput")
    out_d = nc.dram_tensor("out", [1024, 1], F32, kind="ExternalOutput")

    # DRAM scratch: per-block A^T feature tables
    at_dram = [
        nc.dram_tensor(f"at{bi}_scratch", [N, o], F32, kind="Internal")
        for bi, (_, o) in enumerate(BLOCKS)
    ]

    with TileContext(nc) as tc, ExitStack() as ctx:
        ep = ctx.enter_context
        const = ep(tc.tile_pool(name="const", bufs=1))
        wpool = ep(tc.tile_pool(name="weights", bufs=1))
        xpool = ep(tc.tile_pool(name="xtiles", bufs=1))
        spool = ep(tc.tile_pool(name="sbuf", bufs=2))
        tkpool = ep(tc.tile_pool(name="topk", bufs=2))
        gpool = ep(tc.tile_pool(name="gather", bufs=3))
        pp_s = ep(tc.tile_pool(name="ps_s", bufs=4, space="PSUM"))   # [128,512] -> 4 banks
        pp_m = ep(tc.tile_pool(name="ps_m", bufs=2, space="PSUM"))   # small tiles

        # ---- constants ----
        ident = const.tile([P, P], F32)
        nc.sync.dma_start(out=ident[:], in_=id_in[:])
        ones_col = const.tile([P, 1], F32)
        nc.vector.memset(ones_col[:], 1.0)
        ones_row = const.tile([1, P], F32)
        nc.vector.memset(ones_row[:], 1.0)

        # ---- weights (host supplies WnT/WdT/WgT) ----
        wnT, wdT = [], []
        for bi, (c, o) in enumerate(BLOCKS):
            wn = wpool.tile([c, o], F32, tag=f"wn{bi}")
            nc.sync.dma_start(out=wn[:], in_=w_in[f"wnt{bi}"][:])
            wd = wpool.tile([c, o], F32, tag=f"wd{bi}")
            nc.sync.dma_start(out=wd[:], in_=w_in[f"wdt{bi}"][:])
            wnT.append(wn)
            wdT.append(wd)
        wgT = [wpool.tile([P, 1024], F32, tag=f"wg{k}", name=f"wgT{k}")
               for k in range(4)]
        for k in range(4):
            nc.sync.dma_start(out=wgT[k][:], in_=w_in["wgt"][k * P:(k + 1) * P, :])
        # k=0 rows 64:128 again at base partition 0 (x2t half-contraction)
        wg0b = wpool.tile([64, 1024], F32)
        nc.sync.dma_start(out=wg0b[:], in_=w_in["wgt"][64:128, :])

        # ---- persistent feature tiles ----
        # x1t/x2t rows 0:64 = x1/x2, row 64 = ones (fused gram lhs for blocks 2,3)
        x1t = xpool.tile([65, N], F32)
        x2t = xpool.tile([65, N], F32)
        c3 = xpool.tile([P, N], F32)      # x3
        c4a = xpool.tile([P, N], F32)     # x4 rows 0:128
        c4b = xpool.tile([P, N], F32)     # x4 rows 128:256
        gr2 = xpool.tile([65, N], F32)    # rows 0:64 = 2*x1, row 64 = -|x1|^2
        gr3 = xpool.tile([65, N], F32)    # rows 0:64 = 2*x2, row 64 = -|x2|^2
        x24 = xpool.tile([P, N], F32)     # 2*x3
        negxx4 = xpool.tile([1, N], F32)  # -|x3|^2
        redv = xpool.tile([P, 32], F32)   # global-conv per-(oc,nq) maxima
        nc.vector.memset(x1t[64:65, :], 1.0)
        nc.vector.memset(x2t[64:65, :], 1.0)

        # ---- block-1 prep (from the input x0) ----
        # augmented gram operands: gl1 = [x0; 0-pad; ones], gr1 = [2x0; 0-pad; -|x0|^2]
        gl1 = xpool.tile([33, N], F32)
        gr1 = xpool.tile([33, N], F32)
        nc.vector.memset(gl1[:], 0.0)
        nc.vector.memset(gr1[:], 0.0)
        nc.sync.dma_start(out=gl1[0:3, :], in_=x_in[:])
        nc.vector.memset(gl1[32:33, :], 1.0)
        nc.scalar.activation(out=gr1[0:3, :], in_=gl1[0:3, :], func=ACT.Copy, scale=2.0)
        for q in range(4):
            sqq = spool.tile([3, 512], F32, tag="sqq")
            nc.scalar.activation(out=sqq[:], in_=gl1[0:3, q * 512:(q + 1) * 512],
                                 func=ACT.Square)
            mq = pp_m.tile([1, 512], F32, space="PSUM", tag="tp")
            nc.tensor.matmul(out=mq[:], lhsT=ones_col[:3, :], rhs=sqq[:],
                             start=True, stop=True)
            nx0 = spool.tile([1, 512], F32, tag="nx0")
            nc.scalar.activation(out=nx0[:], in_=mq[:], func=ACT.Copy, scale=-1.0)
            nc.sync.dma_start(out=gr1[32:33, q * 512:(q + 1) * 512], in_=nx0[:])
        for i in range(NCHUNK):
            pa = pp_m.tile([P, 64], F32, space="PSUM", tag="mm")
            nc.tensor.matmul(out=pa[:], lhsT=gl1[0:3, i * P:(i + 1) * P], rhs=wnT[0][:],
                             start=True, stop=True)
            at_sb = spool.tile([P, 64], F32, tag="at_sb")
            nc.scalar.copy(out=at_sb[:], in_=pa[:])
            nc.sync.dma_start(out=at_dram[0][i * P:(i + 1) * P, :], in_=at_sb[:])

        # per-block gram operand config
        #   fused: (lhs_tile, lhs_rows, rhs_tile)     two-matmul: (lhs, C, rhs2x, negxx)
        gram_cfg = [
            ("fused", gl1, 33, gr1, None),
            ("fused", x1t, 65, gr2, None),
            ("fused", x2t, 65, gr3, None),
            ("two", c3, P, x24, negxx4),
        ]
        # B-stage destinations + next-block prep config per block
        #   (dsts, nxt_main, nxt_gr2x, nxt_negxx_row_tile_or_None, nxt_at_lhs, nxt_wn)
        tail_cfg = [
            ([x1t], gr2, True, wnT[1]),
            ([x2t], gr3, True, wnT[2]),
            ([c3], None, False, wnT[3]),   # writes x24/negxx4 directly
            ([c4a, c4b], None, False, None),
        ]
        xcat = [None, c3, c4a, c4b]       # k=0 handled as two half-contractions

        for bi, (C, O) in enumerate(BLOCKS):
            mode, glhs, grows, grhs, gneg = gram_cfg[bi]
            st = {}

            def emit_a1(i, bi=bi, C=C, O=O, mode=mode, glhs=glhs, grows=grows,
                        grhs=grhs, gneg=gneg, st=st):
                s_sb = tkpool.tile([P, N], F32, tag="s_sb")
                for q in range(4):
                    sl = slice(q * 512, (q + 1) * 512)
                    ph = pp_s.tile([P, 512], F32, space="PSUM", tag="ph")
                    if mode == "fused":
                        nc.tensor.matmul(out=ph[:], lhsT=glhs[:grows, i * P:(i + 1) * P],
                                         rhs=grhs[:, sl], start=True, stop=True)
                    else:
                        nc.tensor.matmul(out=ph[:], lhsT=glhs[:grows, i * P:(i + 1) * P],
                                         rhs=grhs[:, sl], start=True, stop=False)
                        nc.tensor.matmul(out=ph[:], lhsT=ones_row[:, :P],
                                         rhs=gneg[:, sl], start=False, stop=True)
                    nc.scalar.copy(out=s_sb[:, sl], in_=ph[:])
                st[i] = {"s_sb": s_sb}

            def emit_a2(i, bi=bi, O=O, st=st):
                s_sb = st[i]["s_sb"]
                v8 = tkpool.tile([P, 8], F32, tag="v8")
                i24 = tkpool.tile([P, 24], U16, tag="i24")
                nc.vector.max(out=v8[:], in_=s_sb[:])
                nc.vector.max_index(out=i24[:, 0:8], in_max=v8[:], in_values=s_sb[:])
                nc.vector.match_replace(out=s_sb[:], in_to_replace=v8[:],
                                        in_values=s_sb[:], imm_value=NEG)
                nc.vector.max(out=v8[:], in_=s_sb[:])
                nc.vector.max_index(out=i24[:, 8:16], in_max=v8[:], in_values=s_sb[:])
                nc.vector.match_replace(out=s_sb[:], in_to_replace=v8[:],
                                        in_values=s_sb[:], imm_value=NEG)
                nc.vector.max(out=v8[:], in_=s_sb[:])
                nc.vector.max_index(out=i24[:, 16:24], in_max=v8[:], in_values=s_sb[:])

                nbr32 = tkpool.tile([P, KNN], U32, tag="nbr32")
                nc.vector.tensor_copy(out=nbr32[:], in_=i24[:, :KNN])
                gath = gpool.tile([P, KNN, O], F32, tag="gath")
                for j in range(KNN):
                    nc.gpsimd.indirect_dma_start(
                        out=gath[:, j, :], out_offset=None, in_=at_dram[bi][:],
                        in_offset=IndirectOffsetOnAxis(ap=nbr32[:, j:j + 1], axis=0))
                st[i]["gath"] = gath

            def emit_b(i, bi=bi, C=C, O=O, st=st):
                gath = st.pop(i)["gath"]
                mx = spool.tile([P, O], F32, tag="mx")
                nc.vector.tensor_reduce(
                    out=mx[:], in_=gath[:].rearrange("p k o -> p o k"),
                    axis=mybir.AxisListType.X, op=mybir.AluOpType.max)
                bt = pp_m.tile([P, O], F32, space="PSUM", tag="mm")
                nc.tensor.matmul(out=bt[:], lhsT=gram_cfg[bi][1][:C, i * P:(i + 1) * P],
                                 rhs=wdT[bi][:], start=True, stop=True)
                xt = spool.tile([P, O], F32, tag="xt")
                nc.vector.tensor_add(out=xt[:], in0=mx[:], in1=bt[:])
                xs = spool.tile([P, O], F32, tag="xs")
                nc.vector.tensor_scalar_mul(xs[:], xt[:], SLOPE)
                nc.vector.tensor_tensor(out=xt[:], in0=xt[:], in1=xs[:],
                                        op=mybir.AluOpType.max)

                dsts, nxt_gr, gr_dma, nxt_wn = tail_cfg[bi]
                cols = slice(i * P, (i + 1) * P)
                for q in range((O + P - 1) // P):
                    osz = min(P, O - q * P)
                    tp = pp_m.tile([P, P], F32, space="PSUM", tag="tp")
                    nc.tensor.transpose(out=tp[:osz, :], in_=xt[:, q * P:q * P + osz],
                                        identity=ident[:])
                    nc.scalar.copy(out=dsts[q][0:osz, cols], in_=tp[:osz, :])
                    if bi <= 2:
                        # next-block prep, fused on this chunk's columns
                        tgt2x = nxt_gr[0:osz, cols] if bi <= 1 else x24[0:osz, cols]
                        nc.scalar.activation(out=tgt2x, in_=tp[:osz, :],
                                             func=ACT.Copy, scale=2.0)
                        sqt = spool.tile([O, P], F32, tag="sqt")
                        nc.scalar.activation(out=sqt[:osz, :], in_=tp[:osz, :],
                                             func=ACT.Square)
                if bi <= 2:
                    pn = pp_m.tile([1, P], F32, space="PSUM", tag="tp")
                    nc.tensor.matmul(out=pn[:], lhsT=ones_col[:O, :], rhs=sqt[:],
                                     start=True, stop=True)
                    if bi <= 1:
                        nx = spool.tile([1, P], F32, tag="nx")
                        nc.scalar.activation(out=nx[:], in_=pn[:], func=ACT.Copy,
                                             scale=-1.0)
                        nc.sync.dma_start(out=nxt_gr[64:65, cols], in_=nx[:])
                    else:
                        nc.scalar.activation(out=negxx4[:, cols], in_=pn[:],
                                             func=ACT.Copy, scale=-1.0)
                    # next-block A^T table chunk
                    O2 = BLOCKS[bi + 1][1]
                    pa = pp_m.tile([P, O2], F32, space="PSUM", tag="mm")
                    nc.tensor.matmul(out=pa[:], lhsT=dsts[0][0:O, cols], rhs=nxt_wn[:],
                                     start=True, stop=True)
                    at_sb = spool.tile([P, O2], F32, tag="at_sb")
                    nc.scalar.copy(out=at_sb[:], in_=pa[:])
                    nc.sync.dma_start(out=at_dram[bi + 1][cols, :], in_=at_sb[:])
                if bi == 3 and i % 4 == 3:
                    emit_gquarter(i // 4)

            def emit_gquarter(nq):
                ncols = slice(nq * 512, (nq + 1) * 512)
                for oc in range(8):
                    ocs = slice(oc * P, (oc + 1) * P)
                    pg = pp_s.tile([P, 512], F32, space="PSUM", tag="ph")
                    nc.tensor.matmul(out=pg[:], lhsT=wgT[0][0:64, ocs],
                                     rhs=x1t[0:64, ncols], start=True, stop=False)
                    nc.tensor.matmul(out=pg[:], lhsT=wg0b[:, ocs],
                                     rhs=x2t[0:64, ncols], start=False, stop=False)
                    for k in range(1, 4):
                        nc.tensor.matmul(out=pg[:], lhsT=wgT[k][:, ocs],
                                         rhs=xcat[k][:, ncols],
                                         start=False, stop=(k == 3))
                    nc.vector.tensor_reduce(out=redv[:, oc * 4 + nq:oc * 4 + nq + 1],
                                            in_=pg[:], axis=mybir.AxisListType.X,
                                            op=mybir.AluOpType.max)

            emit_a1(0)
            for i in range(NCHUNK):
                if i + 1 < NCHUNK:
                    emit_a1(i + 1)
                emit_a2(i)
                if i >= 2:
                    emit_b(i - 2)
            emit_b(NCHUNK - 2)
            emit_b(NCHUNK - 1)

        # ---- final: max over the 4 quarters, lrelu, store ----
        for oc in range(8):
            red1 = spool.tile([P, 1], F32, tag="red1")
            nc.vector.tensor_reduce(out=red1[:], in_=redv[:, oc * 4:(oc + 1) * 4],
                                    axis=mybir.AxisListType.X, op=mybir.AluOpType.max)
            reds = spool.tile([P, 1], F32, tag="reds")
            nc.vector.tensor_scalar_mul(reds[:], red1[:], SLOPE)
            nc.vector.tensor_tensor(out=red1[:], in0=red1[:], in1=reds[:],
                                    op=mybir.AluOpType.max)
            nc.sync.dma_start(out=out_d[oc * P:(oc + 1) * P, :], in_=red1[:])

    nc.compile()
    return nc


_NC_CACHE = None


def _get_nc():
    global _NC_CACHE
    if _NC_CACHE is None:
        _NC_CACHE = build_nc()
    return _NC_CACHE


def host_weights(W1, W2, W3, W4, Wg):
    ws = {}
    for bi, (wm, (c, o)) in enumerate(zip([W1, W2, W3, W4], BLOCKS)):
        wm = np.asarray(wm, dtype=np.float32)
        wn = wm[:, :c]
        wd = wm[:, c:] - wn
        ws[f"wnt{bi}"] = np.ascontiguousarray(wn.T)
        ws[f"wdt{bi}"] = np.ascontiguousarray(wd.T)
    ws["wgt"] = np.ascontiguousarray(np.asarray(Wg, dtype=np.float32).T)
    ws["ident"] = np.eye(P, dtype=np.float32)
    return ws


def kernel(x, W1, W2, W3, W4, Wg):
    from concourse.bass_utils import run_bass_kernel_spmd

    nc = _get_nc()
    x = np.asarray(x, dtype=np.float32)
    ws = host_weights(W1, W2, W3, W4, Wg)
    in_maps = [{"x": np.ascontiguousarray(x[b]), **ws} for b in range(B)]
    res = run_bass_kernel_spmd(nc, in_maps, core_ids=list(range(B)))
    outs = res.results if hasattr(res, "results") else res
    return np.stack([outs[b]["out"].reshape(1024) for b in range(B)], axis=0)
